# revision 7
# baseline (speedup 1.0000x reference)
"""GQA attention (B=2, S=2048, D=2048, 32 Q heads / 8 KV heads, HD=64, RoPE,
causal) on 8 TRN2 NeuronCores.

Sharding: tensor-parallel over heads. Core c owns q heads [4c, 4c+4) and kv
head c (GQA groups align exactly with 8 cores); both batches replicated.
Per core:
  - qT/kT/vT projections computed in transposed layout (weights stationary,
    x^T streamed), RoPE applied on-chip via even/odd-split DVE ops.
  - attention in transposed-scores layout s^T[keys, q]: softmax denominators
    come free from an appended all-ones block in the PV matmul; the causal
    mask is preloaded into PSUM via an identity matmul (-1e9 bias -> exp
    gives exact zeros); exp runs on ACT straight from PSUM.
  - per-batch AllGather of the attention-output shards (bf16, ~1MB/rank),
    overlapped with the other batch's attention / wo; then a column-sharded
    wo matmul. Host concatenates + transposes back.
Compute dtype: bf16 matmul operands, fp32 PSUM accumulation, fp32 softmax.
"""

import numpy as np

B, S, D = 2, 2048, 2048
H, KVH, HD = 32, 8, 64
NCORES = 8
BS = B * S            # 4096
NHL = H // NCORES     # 4 q heads per core
MQ = NHL * HD         # 256 q columns per core
SBLK = 512
NSBLK = BS // SBLK    # 8
DC = D // 128         # 16 contraction chunks
NKCH = S // 128       # 16 key chunks per batch


def build_graph(tc, out_ap, xT, wqEO, kvw, woS, c4, s4):
    """Build the per-core SPMD graph. All args are DRAM access patterns."""
    import concourse.mybir as mybir
    from concourse.masks import make_identity

    nc = tc.nc
    F32 = mybir.dt.float32
    BF16 = mybir.dt.bfloat16
    Alu = mybir.AluOpType
    Act = mybir.ActivationFunctionType
    TT = nc.vector.tensor_tensor
    CP = nc.vector.tensor_copy

    with tc.tile_pool(name="const", bufs=1) as constp, \
         tc.tile_pool(name="persist", bufs=1) as pers, \
         tc.tile_pool(name="dram", bufs=1, space="DRAM") as dramp:

        ident = constp.tile([128, 128], F32)
        make_identity(nc, ident[:])
        ident_bf = constp.tile([128, 128], BF16)
        CP(ident_bf[:], ident[:])
        # causal bias tiles: maskT[p, r, col] = -1e9 where col < p + 128r
        maskf = constp.tile([128, 4, 512], F32)
        nc.gpsimd.memset(maskf[:], 0.0)
        for r_ in range(4):
            nc.gpsimd.affine_select(
                out=maskf[:, r_, :], in_=maskf[:, r_, :],
                compare_op=Alu.is_ge, fill=-1e9,
                base=-128 * r_, channel_multiplier=-1, pattern=[[1, 512]])
        maskT = constp.tile([128, 4, 512], BF16)
        CP(maskT[:], maskf[:])

        wq_sb = constp.tile([128, DC, MQ], BF16)
        nc.sync.dma_start(wq_sb[:], wqEO.rearrange("(dc p) m -> p dc m", p=128))
        kvw_sb = constp.tile([128, DC, 128], BF16)
        nc.sync.dma_start(kvw_sb[:], kvw.rearrange("(dc p) m -> p dc m", p=128))

        # persistent activations (bf16)
        qt0 = pers.tile([128, BS], BF16)   # heads 0,1 (rows [0:64], [64:128])
        qt1 = pers.tile([128, BS], BF16)   # heads 2,3
        kT2 = pers.tile([128, BS], BF16)   # kT duplicated at base 0 and base 64
        v1 = pers.tile([128, B * NKCH, 128], BF16)  # [v | ones] per key chunk
        attnT0 = pers.tile([128, BS], BF16)
        attnT1 = pers.tile([128, BS], BF16)
        qts = [qt0, qt1]
        attnTs = [attnT0, attnT1]

        nc.gpsimd.memset(v1[:, :, 64:128], 1.0)

        # ---------------- Phase A: projections + RoPE + v transpose --------
        with tc.tile_pool(name="trig", bufs=1) as trigp, \
             tc.tile_pool(name="xtp", bufs=8) as xtp, \
             tc.tile_pool(name="ropep", bufs=2) as rp, \
             tc.tile_pool(name="psA", bufs=2, space="PSUM") as psA, \
             tc.tile_pool(name="psT", bufs=1, space="PSUM") as psT:
            c4_sb = trigp.tile([128, S], F32)
            nc.sync.dma_start(c4_sb[:], c4[:])
            s4_sb = trigp.tile([128, S], F32)
            nc.sync.dma_start(s4_sb[:], s4[:])
            xTr = xT.rearrange("(dc p) s -> p dc s", p=128)
            for sb in range(NSBLK):
                scol = slice(sb * SBLK, (sb + 1) * SBLK)
                pbeg = (sb % (S // SBLK)) * SBLK
                # split the x^T block into 4 dc-groups so matmuls can start
                # as soon as the first quarter lands
                xts = []
                for g in range(4):
                    xt = xtp.tile([128, 4, SBLK], BF16, tag="xt", name=f"xt{g}")
                    nc.sync.dma_start(xt[:], xTr[:, 4 * g:4 * g + 4, scol])
                    xts.append(xt)
                qE_p = psA.tile([128, SBLK], F32, tag="qE")
                qO_p = psA.tile([128, SBLK], F32, tag="qO")
                kv_p = psA.tile([128, SBLK], F32, tag="kv")
                for dc in range(DC):
                    nc.tensor.matmul(qE_p[:], wq_sb[:, dc, 0:128],
                                     xts[dc // 4][:, dc % 4],
                                     start=(dc == 0), stop=(dc == DC - 1))
                for dc in range(DC):
                    nc.tensor.matmul(qO_p[:], wq_sb[:, dc, 128:256],
                                     xts[dc // 4][:, dc % 4],
                                     start=(dc == 0), stop=(dc == DC - 1))
                for dc in range(DC):
                    nc.tensor.matmul(kv_p[:], kvw_sb[:, dc, :],
                                     xts[dc // 4][:, dc % 4],
                                     start=(dc == 0), stop=(dc == DC - 1))
                c_ = c4_sb[:, pbeg:pbeg + SBLK]
                s_ = s4_sb[:, pbeg:pbeg + SBLK]
                # q RoPE: qE_p rows = 4 heads x 32 even comps, qO_p odd comps
                m1 = rp.tile([128, SBLK], F32, tag="m1")
                TT(m1[:], qE_p[:], c_, Alu.mult)
                m2 = rp.tile([128, SBLK], F32, tag="m2")
                TT(m2[:], qO_p[:], s_, Alu.mult)
                m3 = rp.tile([128, SBLK], F32, tag="m3")
                TT(m3[:], qO_p[:], c_, Alu.mult)
                m4 = rp.tile([128, SBLK], F32, tag="m4")
                TT(m4[:], qE_p[:], s_, Alu.mult)
                oE = rp.tile([128, SBLK], BF16, tag="oE")
                TT(oE[:], m1[:], m2[:], Alu.subtract)
                oO = rp.tile([128, SBLK], BF16, tag="oO")
                TT(oO[:], m3[:], m4[:], Alu.add)
                for h in range(NHL):
                    t, j = h // 2, h % 2
                    CP(qts[t][64 * j:64 * j + 32, scol], oE[32 * h:32 * h + 32, :])
                    CP(qts[t][64 * j + 32:64 * j + 64, scol], oO[32 * h:32 * h + 32, :])
                # k RoPE: kv_p rows [0:32]=kE, [32:64]=kO, [64:128]=vT
                a1 = rp.tile([32, SBLK], F32, tag="a1")
                TT(a1[:], kv_p[0:32, :], c_[0:32, :], Alu.mult)
                b1 = rp.tile([32, SBLK], F32, tag="b1")
                TT(b1[:], kv_p[32:64, :], s_[0:32, :], Alu.mult)
                a2 = rp.tile([32, SBLK], F32, tag="a2")
                TT(a2[:], kv_p[32:64, :], c_[0:32, :], Alu.mult)
                b2 = rp.tile([32, SBLK], F32, tag="b2")
                TT(b2[:], kv_p[0:32, :], s_[0:32, :], Alu.mult)
                TT(kT2[0:32, scol], a1[:], b1[:], Alu.subtract)
                TT(kT2[32:64, scol], a2[:], b2[:], Alu.add)
                CP(kT2[64:96, scol], kT2[0:32, scol])
                CP(kT2[96:128, scol], kT2[32:64, scol])
                # v: evacuate vT window, PE-transpose to natural layout
                vtw = rp.tile([64, SBLK], F32, tag="vtw")
                CP(vtw[:], kv_p[64:128, :])
                for q in range(SBLK // 128):
                    ch = sb * (SBLK // 128) + q
                    tp = psT.tile([128, 64], F32, tag="tp")
                    nc.tensor.transpose(tp[:], vtw[:, q * 128:(q + 1) * 128],
                                        ident[0:64, 0:64])
                    CP(v1[:, ch, 0:64], tp[:])

        # ---------------- Phase B + C interleaved ---------------------------
        attnT_loc = [dramp.tile([MQ, S], BF16, name=f"attnT_loc{b}")
                     for b in range(B)]
        attnT_all = [dramp.tile([D, S], BF16, addr_space="Shared",
                                name=f"attnT_all{b}") for b in range(B)]

        with tc.tile_pool(name="ptp", bufs=2) as ptp, \
             tc.tile_pool(name="recp", bufs=2) as recp, \
             tc.tile_pool(name="wop", bufs=1) as wop, \
             tc.tile_pool(name="attsp", bufs=2) as attsp, \
             tc.tile_pool(name="outsp", bufs=2) as outsp:
            wo_sb = wop.tile([128, DC, MQ], BF16)
            nc.sync.dma_start(wo_sb[:], woS.rearrange("(dc p) m -> p dc m", p=128))
            with tc.tile_pool(name="psS", bufs=1, space="PSUM") as psS, \
                 tc.tile_pool(name="psO", bufs=1, space="PSUM") as psO:
                # attention: heads processed in base-0/base-64 pairs so the
                # two K=64 score matmuls run concurrently on disjoint
                # row-groups of the PE array
                for b in range(B):
                    for t in range(2):
                        rA, rB = slice(0, 64), slice(64, 128)
                        for qh in range(2):
                            oP = [psO.tile([128, 1024], F32, tag=f"o{i}", name=f"o{i}")
                                  for i in range(2)]
                            kmax = 8 * (qh + 1)
                            for k in range(kmax):
                                qf = k // 4
                                r = k - 4 * qf
                                pP = [ptp.tile([128, 1024], BF16, tag=f"p{i}",
                                               name=f"p{i}_{k}")
                                      for i in range(2)]
                                # zero masked prefix [512*qf, 128*k) of window
                                pres = max(512 * qf, 1024 * qh) - 1024 * qh
                                pree = 128 * k - 1024 * qh
                                if pree > pres:
                                    for p_ in pP:
                                        nc.gpsimd.memset(p_[:, pres:pree], 0.0)
                                sP = [psS.tile([128, 1024], F32, tag=f"s{i}", name=f"s{i}")
                                      for i in range(2)]
                                for qsub in (2 * qh, 2 * qh + 1):
                                    if qsub < qf:
                                        continue
                                    o = (qsub - 2 * qh) * 512
                                    if qsub == qf:
                                        # causal bias preload (full bank)
                                        for s_ in sP:
                                            nc.tensor.matmul(
                                                s_[:, o:o + 512],
                                                ident_bf[:], maskT[:, r, :],
                                                start=True, stop=False)
                                    kc = slice(b * S + k * 128, b * S + k * 128 + 128)
                                    qc = slice(b * S + qsub * 512, b * S + qsub * 512 + 512)
                                    nc.tensor.matmul(
                                        sP[0][:, o:o + 512], kT2[rA, kc],
                                        qts[t][rA, qc], start=(qsub != qf), stop=True)
                                    nc.tensor.matmul(
                                        sP[1][:, o:o + 512], kT2[rB, kc],
                                        qts[t][rB, qc], start=(qsub != qf), stop=True)
                                es = max(0, 128 * k - 1024 * qh)
                                for s_, p_ in zip(sP, pP):
                                    nc.scalar.activation(
                                        p_[:, es:1024], s_[:, es:1024],
                                        Act.Exp, scale=0.125)
                                for qsub in (2 * qh, 2 * qh + 1):
                                    if qsub < qf:
                                        continue
                                    o = (qsub - 2 * qh) * 512
                                    last_k = min(kmax - 1, 4 * qsub + 3)
                                    for p_, o_ in zip(pP, oP):
                                        nc.tensor.matmul(
                                            o_[:, o:o + 512],
                                            v1[:, b * NKCH + k, :],
                                            p_[:, o:o + 512],
                                            start=(k == 0), stop=(k == last_k))
                            # normalize: rows [64:128] hold the denominator
                            qcg = slice(b * S + 1024 * qh, b * S + 1024 * qh + 1024)
                            for i, (o_, rows) in enumerate(zip(oP, (rA, rB))):
                                den = recp.tile([64, 1024], F32, tag=f"den{i}", name=f"den{i}")
                                CP(den[:], o_[64:128, :])
                                rec = recp.tile([64, 1024], F32, tag=f"rec{i}", name=f"rec{i}")
                                nc.vector.reciprocal_approx_fast(rec[:], den[:])
                                TT(attnTs[t][rows, qcg], o_[0:64, :], rec[:],
                                   Alu.mult)
                    # ship this batch's attn output and start its AllGather,
                    # overlapping the next batch's attention
                    bc = slice(b * S, (b + 1) * S)
                    nc.sync.dma_start(attnT_loc[b][0:128, :], attnT0[:, bc])
                    nc.sync.dma_start(attnT_loc[b][128:256, :], attnT1[:, bc])
                    nc.gpsimd.collective_compute(
                        "AllGather", mybir.AluOpType.bypass,
                        replica_groups=[list(range(NCORES))],
                        ins=[attnT_loc[b].opt()], outs=[attnT_all[b].opt()])

            # ---------------- wo (att tiles prefetch during attention) ------
            with tc.tile_pool(name="psW", bufs=4, space="PSUM") as psW:
                for b in range(B):
                    attr = attnT_all[b].rearrange("(dc p) s -> p dc s", p=128)
                    for sbl in range(S // 1024):
                        scol = slice(sbl * 1024, (sbl + 1) * 1024)
                        att = attsp.tile([128, DC, 1024], BF16, tag="att")
                        nc.sync.dma_start(att[:], attr[:, :, scol])
                        for mc in range(MQ // 128):
                            for half in range(2):
                                wp = psW.tile([128, 512], F32, tag="wp")
                                for dc in range(DC):
                                    nc.tensor.matmul(
                                        wp[:], wo_sb[:, dc, mc * 128:(mc + 1) * 128],
                                        att[:, dc, half * 512:half * 512 + 512],
                                        start=(dc == 0), stop=(dc == DC - 1))
                                ot = outsp.tile([128, 512], F32, tag="ot")
                                CP(ot[:], wp[:])
                                nc.sync.dma_start(
                                    out_ap[mc * 128:(mc + 1) * 128,
                                           b * S + sbl * 1024 + half * 512:
                                           b * S + sbl * 1024 + half * 512 + 512],
                                    ot[:])


def prep_inputs(x, cos, sin, wq, wk, wv, wo):
    """Host-side layout prep. Returns per-core input dicts (bf16/f32)."""
    import ml_dtypes
    bf16 = ml_dtypes.bfloat16
    x = np.asarray(x, np.float32)
    cos = np.asarray(cos, np.float32)
    sin = np.asarray(sin, np.float32)
    wq = np.asarray(wq, np.float32)
    wk = np.asarray(wk, np.float32)
    wv = np.asarray(wv, np.float32)
    wo = np.asarray(wo, np.float32)

    xT = np.ascontiguousarray(x.transpose(2, 0, 1).reshape(D, BS)).astype(bf16)
    c4 = np.ascontiguousarray(np.tile(cos.T, (4, 1)))          # [128, S] f32
    s4 = np.ascontiguousarray(np.tile(sin.T, (4, 1)))
    eperm = np.array([64 * h + 2 * j for h in range(NHL) for j in range(32)])
    operm = eperm + 1
    in_maps = []
    for c in range(NCORES):
        wq_sh = wq[:, MQ * c:MQ * c + MQ]
        wqEO = np.concatenate([wq_sh[:, eperm], wq_sh[:, operm]], axis=1)
        kc = wk[:, HD * c:HD * c + HD]
        vc = wv[:, HD * c:HD * c + HD]
        kvw = np.concatenate([kc[:, 0::2], kc[:, 1::2], vc], axis=1)
        woS = wo[:, MQ * c:MQ * c + MQ]
        in_maps.append({
            "xT": xT,
            "wqEO": np.ascontiguousarray(wqEO).astype(bf16),
            "kvw": np.ascontiguousarray(kvw).astype(bf16),
            "woS": np.ascontiguousarray(woS).astype(bf16),
            "c4": c4,
            "s4": s4,
        })
    return in_maps


def assemble_output(core_outs):
    """core_outs: list of 8 [256, BS] f32 arrays -> [B, S, D] f32."""
    outT = np.concatenate(core_outs, axis=0)           # [D, BS]
    return np.ascontiguousarray(
        outT.reshape(D, B, S).transpose(1, 2, 0)).astype(np.float32)


_CACHE = {}


def _get_compiled():
    if "nc" in _CACHE:
        return _CACHE["nc"]
    import concourse.mybir as mybir
    import concourse.tile as tile
    from concourse import bacc

    nc = bacc.Bacc("TRN2", target_bir_lowering=False, debug=False,
                   num_devices=NCORES)
    F32 = mybir.dt.float32
    BF16 = mybir.dt.bfloat16
    xT_d = nc.dram_tensor("xT", [D, BS], BF16, kind="ExternalInput")
    wq_d = nc.dram_tensor("wqEO", [D, MQ], BF16, kind="ExternalInput")
    kvw_d = nc.dram_tensor("kvw", [D, 128], BF16, kind="ExternalInput")
    wo_d = nc.dram_tensor("woS", [D, MQ], BF16, kind="ExternalInput")
    c4_d = nc.dram_tensor("c4", [128, S], F32, kind="ExternalInput")
    s4_d = nc.dram_tensor("s4", [128, S], F32, kind="ExternalInput")
    out_d = nc.dram_tensor("out", [MQ, BS], F32, kind="ExternalOutput")
    with tile.TileContext(nc) as tc:
        build_graph(tc, out_d.ap(), xT_d.ap(), wq_d.ap(), kvw_d.ap(),
                    wo_d.ap(), c4_d.ap(), s4_d.ap())
    nc.compile()
    _CACHE["nc"] = nc
    return nc


def kernel(x, cos, sin, wq, wk, wv, wo):
    from concourse.bass_utils import run_bass_kernel_spmd
    nc = _get_compiled()
    in_maps = prep_inputs(x, cos, sin, wq, wk, wv, wo)
    res = run_bass_kernel_spmd(nc, in_maps, core_ids=list(range(NCORES)))
    _CACHE["last_results"] = res
    return assemble_output([res.results[c]["out"] for c in range(NCORES)])


# revision 10
# speedup vs baseline: 1.0710x; 1.0710x over previous
"""GQA attention (B=2, S=2048, D=2048, 32 Q heads / 8 KV heads, HD=64, RoPE,
causal) on 8 TRN2 NeuronCores.

Sharding: tensor-parallel over heads. Core c owns q heads [4c, 4c+4) and kv
head c (GQA groups align exactly with 8 cores); both batches replicated.
Per core:
  - qT/kT/vT projections computed in transposed layout (weights stationary,
    x^T streamed), RoPE applied on-chip via even/odd-split DVE ops.
  - attention in transposed-scores layout s^T[keys, q]: softmax denominators
    come free from an appended all-ones block in the PV matmul; the causal
    mask is preloaded into PSUM via an identity matmul (-1e9 bias -> exp
    gives exact zeros); exp runs on ACT straight from PSUM.
  - per-batch AllGather of the attention-output shards (bf16, ~1MB/rank),
    overlapped with the other batch's attention / wo; then a column-sharded
    wo matmul. Host concatenates + transposes back.
Compute dtype: bf16 matmul operands, fp32 PSUM accumulation, fp32 softmax.
"""

import numpy as np

B, S, D = 2, 2048, 2048
H, KVH, HD = 32, 8, 64
NCORES = 8
BS = B * S            # 4096
NHL = H // NCORES     # 4 q heads per core
MQ = NHL * HD         # 256 q columns per core
SBLK = 512
NSBLK = BS // SBLK    # 8
DC = D // 128         # 16 contraction chunks
NKCH = S // 128       # 16 key chunks per batch


def build_graph(tc, out_ap, xT, wqEO, kvw, woS, c4, s4):
    """Build the per-core SPMD graph. All args are DRAM access patterns."""
    import concourse.mybir as mybir
    from concourse.masks import make_identity

    nc = tc.nc
    F32 = mybir.dt.float32
    BF16 = mybir.dt.bfloat16
    Alu = mybir.AluOpType
    Act = mybir.ActivationFunctionType
    TT = nc.vector.tensor_tensor
    CP = nc.vector.tensor_copy

    with tc.tile_pool(name="const", bufs=1) as constp, \
         tc.tile_pool(name="persist", bufs=1) as pers, \
         tc.tile_pool(name="dram", bufs=1, space="DRAM") as dramp:

        ident = constp.tile([128, 128], F32)
        make_identity(nc, ident[:])
        ident_bf = constp.tile([128, 128], BF16)
        CP(ident_bf[:], ident[:])
        wq_sb = constp.tile([128, DC, MQ], BF16)
        nc.sync.dma_start(wq_sb[:], wqEO.rearrange("(dc p) m -> p dc m", p=128))
        kvw_sb = constp.tile([128, DC, 128], BF16)
        nc.sync.dma_start(kvw_sb[:], kvw.rearrange("(dc p) m -> p dc m", p=128))

        # persistent activations (bf16)
        qt0 = pers.tile([128, BS], BF16)   # heads 0,1 (rows [0:64], [64:128])
        qt1 = pers.tile([128, BS], BF16)   # heads 2,3
        kT2 = pers.tile([128, BS], BF16)   # kT duplicated at base 0 and base 64
        v1 = pers.tile([128, B * NKCH, 128], BF16)  # [v | ones] per key chunk
        attnT0 = pers.tile([128, BS], BF16)
        attnT1 = pers.tile([128, BS], BF16)
        qts = [qt0, qt1]
        attnTs = [attnT0, attnT1]

        nc.gpsimd.memset(v1[:, :, 64:128], 1.0)

        # ---------------- Phase A: projections + RoPE + v transpose --------
        with tc.tile_pool(name="trig", bufs=1) as trigp, \
             tc.tile_pool(name="xtp", bufs=8) as xtp, \
             tc.tile_pool(name="ropep", bufs=2) as rp, \
             tc.tile_pool(name="psA", bufs=2, space="PSUM") as psA, \
             tc.tile_pool(name="psT", bufs=1, space="PSUM") as psT:
            c4_sb = trigp.tile([128, S], F32)
            nc.sync.dma_start(c4_sb[:], c4[:])
            s4_sb = trigp.tile([128, S], F32)
            nc.sync.dma_start(s4_sb[:], s4[:])
            xTr = xT.rearrange("(dc p) s -> p dc s", p=128)
            for sb in range(NSBLK):
                scol = slice(sb * SBLK, (sb + 1) * SBLK)
                pbeg = (sb % (S // SBLK)) * SBLK
                # split the x^T block into 4 dc-groups so matmuls can start
                # as soon as the first quarter lands
                xts = []
                for g in range(4):
                    xt = xtp.tile([128, 4, SBLK], BF16, tag="xt", name=f"xt{g}")
                    nc.sync.dma_start(xt[:], xTr[:, 4 * g:4 * g + 4, scol])
                    xts.append(xt)
                qE_p = psA.tile([128, SBLK], F32, tag="qE")
                qO_p = psA.tile([128, SBLK], F32, tag="qO")
                kv_p = psA.tile([128, SBLK], F32, tag="kv")
                for dc in range(DC):
                    nc.tensor.matmul(qE_p[:], wq_sb[:, dc, 0:128],
                                     xts[dc // 4][:, dc % 4],
                                     start=(dc == 0), stop=(dc == DC - 1))
                for dc in range(DC):
                    nc.tensor.matmul(qO_p[:], wq_sb[:, dc, 128:256],
                                     xts[dc // 4][:, dc % 4],
                                     start=(dc == 0), stop=(dc == DC - 1))
                for dc in range(DC):
                    nc.tensor.matmul(kv_p[:], kvw_sb[:, dc, :],
                                     xts[dc // 4][:, dc % 4],
                                     start=(dc == 0), stop=(dc == DC - 1))
                c_ = c4_sb[:, pbeg:pbeg + SBLK]
                s_ = s4_sb[:, pbeg:pbeg + SBLK]
                # q RoPE: qE_p rows = 4 heads x 32 even comps, qO_p odd comps
                m1 = rp.tile([128, SBLK], F32, tag="m1")
                TT(m1[:], qE_p[:], c_, Alu.mult)
                m2 = rp.tile([128, SBLK], F32, tag="m2")
                TT(m2[:], qO_p[:], s_, Alu.mult)
                m3 = rp.tile([128, SBLK], F32, tag="m3")
                TT(m3[:], qO_p[:], c_, Alu.mult)
                m4 = rp.tile([128, SBLK], F32, tag="m4")
                TT(m4[:], qE_p[:], s_, Alu.mult)
                oE = rp.tile([128, SBLK], BF16, tag="oE")
                TT(oE[:], m1[:], m2[:], Alu.subtract)
                oO = rp.tile([128, SBLK], BF16, tag="oO")
                TT(oO[:], m3[:], m4[:], Alu.add)
                for h in range(NHL):
                    t, j = h // 2, h % 2
                    CP(qts[t][64 * j:64 * j + 32, scol], oE[32 * h:32 * h + 32, :])
                    CP(qts[t][64 * j + 32:64 * j + 64, scol], oO[32 * h:32 * h + 32, :])
                # k RoPE: kv_p rows [0:32]=kE, [32:64]=kO, [64:128]=vT
                a1 = rp.tile([32, SBLK], F32, tag="a1")
                TT(a1[:], kv_p[0:32, :], c_[0:32, :], Alu.mult)
                b1 = rp.tile([32, SBLK], F32, tag="b1")
                TT(b1[:], kv_p[32:64, :], s_[0:32, :], Alu.mult)
                a2 = rp.tile([32, SBLK], F32, tag="a2")
                TT(a2[:], kv_p[32:64, :], c_[0:32, :], Alu.mult)
                b2 = rp.tile([32, SBLK], F32, tag="b2")
                TT(b2[:], kv_p[0:32, :], s_[0:32, :], Alu.mult)
                TT(kT2[0:32, scol], a1[:], b1[:], Alu.subtract)
                TT(kT2[32:64, scol], a2[:], b2[:], Alu.add)
                CP(kT2[64:96, scol], kT2[0:32, scol])
                CP(kT2[96:128, scol], kT2[32:64, scol])
                # v: evacuate vT window, PE-transpose to natural layout
                vtw = rp.tile([64, SBLK], F32, tag="vtw")
                CP(vtw[:], kv_p[64:128, :])
                for q in range(SBLK // 128):
                    ch = sb * (SBLK // 128) + q
                    tp = psT.tile([128, 64], F32, tag="tp")
                    nc.tensor.transpose(tp[:], vtw[:, q * 128:(q + 1) * 128],
                                        ident[0:64, 0:64])
                    CP(v1[:, ch, 0:64], tp[:])

        # ---------------- Phase B + C interleaved ---------------------------
        attnT_loc = [dramp.tile([MQ, S], BF16, name=f"attnT_loc{b}")
                     for b in range(B)]
        attnT_all = [dramp.tile([D, S], BF16, addr_space="Shared",
                                name=f"attnT_all{b}") for b in range(B)]

        with tc.tile_pool(name="ptp", bufs=2) as ptp, \
             tc.tile_pool(name="recp", bufs=2) as recp, \
             tc.tile_pool(name="wop", bufs=1) as wop, \
             tc.tile_pool(name="attsp", bufs=2) as attsp, \
             tc.tile_pool(name="outsp", bufs=2) as outsp:
            wo_sb = wop.tile([128, DC, MQ], BF16)
            nc.sync.dma_start(wo_sb[:], woS.rearrange("(dc p) m -> p dc m", p=128))
            with tc.tile_pool(name="psS", bufs=1, space="PSUM") as psS, \
                 tc.tile_pool(name="psO", bufs=1, space="PSUM") as psO:
                # attention: heads processed in base-0/base-64 pairs so the
                # two K=64 score matmuls run concurrently on disjoint
                # row-groups of the PE array
                for b in range(B):
                    for t in range(2):
                        rA, rB = slice(0, 64), slice(64, 128)
                        for qh in range(2):
                            oP = [psO.tile([128, 1024], F32, tag=f"o{i}", name=f"o{i}")
                                  for i in range(2)]
                            kmax = 8 * (qh + 1)
                            for k in range(kmax):
                                qf = k // 4
                                r = k - 4 * qf
                                pP = [ptp.tile([128, 1024], BF16, tag=f"p{i}",
                                               name=f"p{i}_{k}")
                                      for i in range(2)]
                                # zero masked prefix [512*qf, 128*k) of window
                                pres = max(512 * qf, 1024 * qh) - 1024 * qh
                                pree = 128 * k - 1024 * qh
                                if pree > pres:
                                    for p_ in pP:
                                        nc.gpsimd.memset(p_[:, pres:pree], 0.0)
                                sP = [psS.tile([128, 1024], F32, tag=f"s{i}", name=f"s{i}")
                                      for i in range(2)]
                                for qsub in (2 * qh, 2 * qh + 1):
                                    if qsub < qf:
                                        continue
                                    o = (qsub - 2 * qh) * 512
                                    kc = slice(b * S + k * 128, b * S + k * 128 + 128)
                                    qc = slice(b * S + qsub * 512, b * S + qsub * 512 + 512)
                                    nc.tensor.matmul(
                                        sP[0][:, o:o + 512], kT2[rA, kc],
                                        qts[t][rA, qc], start=True, stop=True)
                                    nc.tensor.matmul(
                                        sP[1][:, o:o + 512], kT2[rB, kc],
                                        qts[t][rB, qc], start=True, stop=True)
                                es = max(0, 128 * k - 1024 * qh)
                                for s_, p_ in zip(sP, pP):
                                    nc.scalar.activation(
                                        p_[:, es:1024], s_[:, es:1024],
                                        Act.Exp, scale=0.125)
                                if qf // 2 == qh:
                                    # zero the causal triangle (key > q)
                                    for p_ in pP:
                                        nc.gpsimd.affine_select(
                                            out=p_[:, es:es + 128],
                                            in_=p_[:, es:es + 128],
                                            compare_op=Alu.is_ge, fill=0.0,
                                            base=0, channel_multiplier=-1,
                                            pattern=[[1, 128]])
                                for qsub in (2 * qh, 2 * qh + 1):
                                    if qsub < qf:
                                        continue
                                    o = (qsub - 2 * qh) * 512
                                    last_k = min(kmax - 1, 4 * qsub + 3)
                                    for p_, o_ in zip(pP, oP):
                                        nc.tensor.matmul(
                                            o_[:, o:o + 512],
                                            v1[:, b * NKCH + k, :],
                                            p_[:, o:o + 512],
                                            start=(k == 0), stop=(k == last_k))
                            # normalize: rows [64:128] hold the denominator
                            qcg = slice(b * S + 1024 * qh, b * S + 1024 * qh + 1024)
                            for i, (o_, rows) in enumerate(zip(oP, (rA, rB))):
                                raw = recp.tile([128, 1024], F32, tag=f"raw{i}", name=f"raw{i}")
                                CP(raw[:], o_[:])
                                den = recp.tile([64, 1024], F32, tag=f"den{i}", name=f"den{i}")
                                CP(den[:], raw[64:128, :])
                                rec = recp.tile([64, 1024], F32, tag=f"rec{i}", name=f"rec{i}")
                                nc.vector.reciprocal_approx_fast(rec[:], den[:])
                                TT(attnTs[t][rows, qcg], raw[0:64, :], rec[:],
                                   Alu.mult)
                    # ship this batch's attn output and start its AllGather,
                    # overlapping the next batch's attention
                    bc = slice(b * S, (b + 1) * S)
                    nc.sync.dma_start(attnT_loc[b][0:128, :], attnT0[:, bc])
                    nc.sync.dma_start(attnT_loc[b][128:256, :], attnT1[:, bc])
                    nc.gpsimd.collective_compute(
                        "AllGather", mybir.AluOpType.bypass,
                        replica_groups=[list(range(NCORES))],
                        ins=[attnT_loc[b].opt()], outs=[attnT_all[b].opt()])

            # ---------------- wo (att tiles prefetch during attention) ------
            with tc.tile_pool(name="psW", bufs=4, space="PSUM") as psW:
                for b in range(B):
                    attr = attnT_all[b].rearrange("(dc p) s -> p dc s", p=128)
                    for sbl in range(S // 1024):
                        scol = slice(sbl * 1024, (sbl + 1) * 1024)
                        att = attsp.tile([128, DC, 1024], BF16, tag="att")
                        nc.sync.dma_start(att[:], attr[:, :, scol])
                        for mc in range(MQ // 128):
                            for half in range(2):
                                wp = psW.tile([128, 512], F32, tag="wp")
                                for dc in range(DC):
                                    nc.tensor.matmul(
                                        wp[:], wo_sb[:, dc, mc * 128:(mc + 1) * 128],
                                        att[:, dc, half * 512:half * 512 + 512],
                                        start=(dc == 0), stop=(dc == DC - 1))
                                ot = outsp.tile([128, 512], F32, tag="ot")
                                CP(ot[:], wp[:])
                                nc.sync.dma_start(
                                    out_ap[mc * 128:(mc + 1) * 128,
                                           b * S + sbl * 1024 + half * 512:
                                           b * S + sbl * 1024 + half * 512 + 512],
                                    ot[:])


def prep_inputs(x, cos, sin, wq, wk, wv, wo):
    """Host-side layout prep. Returns per-core input dicts (bf16/f32)."""
    import ml_dtypes
    bf16 = ml_dtypes.bfloat16
    x = np.asarray(x, np.float32)
    cos = np.asarray(cos, np.float32)
    sin = np.asarray(sin, np.float32)
    wq = np.asarray(wq, np.float32)
    wk = np.asarray(wk, np.float32)
    wv = np.asarray(wv, np.float32)
    wo = np.asarray(wo, np.float32)

    xT = np.ascontiguousarray(x.transpose(2, 0, 1).reshape(D, BS)).astype(bf16)
    c4 = np.ascontiguousarray(np.tile(cos.T, (4, 1)))          # [128, S] f32
    s4 = np.ascontiguousarray(np.tile(sin.T, (4, 1)))
    eperm = np.array([64 * h + 2 * j for h in range(NHL) for j in range(32)])
    operm = eperm + 1
    in_maps = []
    for c in range(NCORES):
        wq_sh = wq[:, MQ * c:MQ * c + MQ]
        wqEO = np.concatenate([wq_sh[:, eperm], wq_sh[:, operm]], axis=1)
        kc = wk[:, HD * c:HD * c + HD]
        vc = wv[:, HD * c:HD * c + HD]
        kvw = np.concatenate([kc[:, 0::2], kc[:, 1::2], vc], axis=1)
        woS = wo[:, MQ * c:MQ * c + MQ]
        in_maps.append({
            "xT": xT,
            "wqEO": np.ascontiguousarray(wqEO).astype(bf16),
            "kvw": np.ascontiguousarray(kvw).astype(bf16),
            "woS": np.ascontiguousarray(woS).astype(bf16),
            "c4": c4,
            "s4": s4,
        })
    return in_maps


def assemble_output(core_outs):
    """core_outs: list of 8 [256, BS] f32 arrays -> [B, S, D] f32."""
    outT = np.concatenate(core_outs, axis=0)           # [D, BS]
    return np.ascontiguousarray(
        outT.reshape(D, B, S).transpose(1, 2, 0)).astype(np.float32)


_CACHE = {}


def _get_compiled():
    if "nc" in _CACHE:
        return _CACHE["nc"]
    import concourse.mybir as mybir
    import concourse.tile as tile
    from concourse import bacc

    nc = bacc.Bacc("TRN2", target_bir_lowering=False, debug=False,
                   num_devices=NCORES)
    F32 = mybir.dt.float32
    BF16 = mybir.dt.bfloat16
    xT_d = nc.dram_tensor("xT", [D, BS], BF16, kind="ExternalInput")
    wq_d = nc.dram_tensor("wqEO", [D, MQ], BF16, kind="ExternalInput")
    kvw_d = nc.dram_tensor("kvw", [D, 128], BF16, kind="ExternalInput")
    wo_d = nc.dram_tensor("woS", [D, MQ], BF16, kind="ExternalInput")
    c4_d = nc.dram_tensor("c4", [128, S], F32, kind="ExternalInput")
    s4_d = nc.dram_tensor("s4", [128, S], F32, kind="ExternalInput")
    out_d = nc.dram_tensor("out", [MQ, BS], F32, kind="ExternalOutput")
    with tile.TileContext(nc) as tc:
        build_graph(tc, out_d.ap(), xT_d.ap(), wq_d.ap(), kvw_d.ap(),
                    wo_d.ap(), c4_d.ap(), s4_d.ap())
    nc.compile()
    _CACHE["nc"] = nc
    return nc


def kernel(x, cos, sin, wq, wk, wv, wo):
    from concourse.bass_utils import run_bass_kernel_spmd
    nc = _get_compiled()
    in_maps = prep_inputs(x, cos, sin, wq, wk, wv, wo)
    res = run_bass_kernel_spmd(nc, in_maps, core_ids=list(range(NCORES)))
    _CACHE["last_results"] = res
    return assemble_output([res.results[c]["out"] for c in range(NCORES)])
